# revision 7
# baseline (speedup 1.0000x reference)
"""Int8-dequant linear layer (out = input @ (qweight*scale).T + bias) on 8 trn2 cores.

Sharding: token-parallel. input [8,512,4096] flattens to 4096 tokens; each
core computes 512 tokens against the full weight matrix. qweight is repacked
host-side to fp16 (exact: values are integers in [-127,127]) in a
[of_chunk, partition, k_tile, n] layout so each weight DMA is a single
contiguous 4MB read. The per-tensor scale and the bias are applied on-device
in fp32 (ScalarE mul + VectorE add) after fp32 PSUM accumulation, so the only
precision loss vs the fp32 reference is the fp16 rounding of the activations
(~1e-4 relative).
"""

import numpy as np

B, S, IN_F, OUT_F = 8, 512, 4096, 4096
N_CORES = 8
TOK = B * S                # 4096 tokens total
TOK_C = TOK // N_CORES     # 512 tokens per core
P = 128                    # partitions
KT = IN_F // P             # 32 k-tiles
NT = 512                   # out-feature chunk (one fp32 PSUM bank)
OF_CHUNKS = OUT_F // NT    # 8
TT = TOK_C // P            # 4 token tiles per core


def _make_tile_context_cls():
    import bass_rust
    import concourse.mybir as mybir
    from concourse.tile import TileContext, ScopedClock

    class _TC(TileContext):
        # The walrus build in this image rejects more than one semaphore wait
        # per instruction. Split extra waits onto nofuse NOPs committed just
        # before the instruction on the same engine (identical queue
        # semantics: the sequencer blocks on the NOP's wait first).
        def _commit_instruction(self, inst, lazy_reg_writes: bool = True):
            si = getattr(inst, "sync_info", None)
            if (
                si is not None
                and len(si.on_wait) > 1
                and inst.engine != mybir.EngineType.Unassigned
            ):
                waits = list(si.on_wait)
                for i, w in enumerate(waits[:-1]):
                    nop = mybir.InstNoOp(
                        name=f"{inst.name}-ws{i}",
                        sync_info=mybir.SyncInfo(on_wait=[w], on_update=[]),
                        bass_nofuse=True,
                        engine=inst.engine,
                    )
                    self._add_instruction(nop)
                inst.sync_info = mybir.SyncInfo(
                    on_wait=[waits[-1]], on_update=list(si.on_update)
                )
            return super()._commit_instruction(inst, lazy_reg_writes)

        # Same walrus limitation: it can't encode syncs on the exit Drain, so
        # land the end-of-kernel clock waits on single-wait NOPs and use the
        # sequencer-level (EVSEM-only) barrier instead of the drain butterfly.
        def _drain_and_barrier(self, tick_clock, wait_clock):
            nc = self.nc
            carrier = nc.sync.nop(nofuse=True)
            wait_clock.add_sem_waits(
                carrier.ins, ScopedClock({None: tick_clock.global_clock})
            )
            waits = list(carrier.ins.sync_info.on_wait)
            if len(waits) > 1:
                carrier.ins.sync_info = bass_rust.SyncInfo(
                    on_wait=[waits[0]], on_update=[]
                )
                for w in waits[1:]:
                    extra = nc.sync.nop(nofuse=True)
                    extra.ins.sync_info = bass_rust.SyncInfo(
                        on_wait=[w], on_update=[]
                    )
            nc.sync.drain()
            nc.all_engine_barrier(sem_only=True)
            assert self.sems is not None
            popped = nc._tile_sem_poison_stack.pop()
            assert popped is self._sem_poison
            nc.clear_and_free_semaphores(list(self.sems.allocated().values()))
            nc.all_engine_barrier(sem_only=True)

    return _TC


def build_nc():
    """Build the per-core Bass program (SPMD: same program, different x shard)."""
    import concourse.bass as bass
    import concourse.mybir as mybir
    from concourse.masks import make_identity

    f16 = mybir.dt.float16
    f32 = mybir.dt.float32

    nc = bass.Bass("TRN2", target_bir_lowering=False, debug=False)
    x = nc.dram_tensor("x", [TOK_C, IN_F], f32, kind="ExternalInput").ap()
    # weights ship as int8 (exact) and are cast to fp16 inline by the SWDGE
    # DMA — halves weight HBM traffic vs fp16-in-DRAM (measured ~19us/pass).
    wt = nc.dram_tensor(
        "wt", [OF_CHUNKS, P, KT, NT], mybir.dt.int8, kind="ExternalInput"
    ).ap()
    # bias comes pre-broadcast to 128 partitions from the host: a plain
    # contiguous 2MB DMA is ~50us faster than a [1,N]->[128,N] broadcast
    # DMA (which re-reads the same 16KB region 128 times).
    bias = nc.dram_tensor("bias", [P, OUT_F], f32, kind="ExternalInput").ap()
    scale = nc.dram_tensor("scale", [1, 1], f32, kind="ExternalInput").ap()
    out = nc.dram_tensor("out", [TOK_C, OUT_F], f32, kind="ExternalOutput").ap()

    TC = _make_tile_context_cls()
    with TC(nc) as tc:
        with (
            tc.tile_pool(name="persist", bufs=1) as persist,
            tc.tile_pool(name="xstage", bufs=2) as xstage,
            tc.tile_pool(name="wpool", bufs=2) as wpool,
            tc.tile_pool(name="opool", bufs=4) as opool,
            tc.tile_pool(name="pt", bufs=4, space="PSUM") as pt_pool,
            tc.tile_pool(name="pacc", bufs=4, space="PSUM") as pacc_pool,
        ):
            identity = persist.tile([P, P], f16)
            make_identity(nc, identity)
            scale_sb = persist.tile([P, 1], f32)
            nc.sync.dma_start(out=scale_sb, in_=scale.to_broadcast((P, 1)))
            bias_sb = persist.tile([P, OUT_F], f32)
            nc.sync.dma_start(out=bias_sb, in_=bias)

            # xt_all[p, j, t] = x[t, j*128+p]  (fp16 transposed activations)
            xt_all = persist.tile([P, KT, TOK_C], f16)

            # Phase 1: load x (cast fp32->fp16 in DMA), transpose on PE.
            for t in range(TT):
                xs = xstage.tile([P, IN_F], f16)
                nc.gpsimd.dma_start(out=xs, in_=x[t * P:(t + 1) * P, :])
                for j in range(KT):
                    ps = pt_pool.tile([P, P], f16)
                    nc.tensor.transpose(ps, xs[:, j * P:(j + 1) * P], identity)
                    nc.vector.tensor_copy(xt_all[:, j, t * P:(t + 1) * P], ps)

            # Phase 2: stream weight chunks, accumulate, epilogue, store.
            for of in range(OF_CHUNKS):
                wc = wpool.tile([P, KT, NT], f16)
                nc.gpsimd.dma_start(out=wc, in_=wt[of])  # int8 -> fp16 cast
                for t in range(TT):
                    acc = pacc_pool.tile([P, NT], f32)
                    for j in range(KT):
                        nc.tensor.matmul(
                            acc,
                            lhsT=xt_all[:, j, t * P:(t + 1) * P],
                            rhs=wc[:, j, :],
                            start=(j == 0),
                            stop=(j == KT - 1),
                        )
                    osb = opool.tile([P, NT], f32)
                    nc.scalar.mul(osb, acc, scale_sb[:, :])
                    nc.vector.tensor_add(osb, osb, bias_sb[:, of * NT:(of + 1) * NT])
                    nc.sync.dma_start(
                        out=out[t * P:(t + 1) * P, of * NT:(of + 1) * NT], in_=osb
                    )
    return nc


def prep_inputs(input, qweight, weight_scale, bias_param):
    """Host-side shard/repack. Returns per-core in_maps."""
    X = np.ascontiguousarray(np.asarray(input, dtype=np.float32).reshape(TOK, IN_F))
    # int8 container for the int8-valued weights; the device DMA casts to fp16
    # (exact for integers in [-127,127]).
    q8 = np.asarray(qweight).astype(np.int8)
    # w_packed[of, p, j, n] = qweight[of*NT + n, j*P + p]
    wp = np.ascontiguousarray(
        q8.reshape(OF_CHUNKS, NT, KT, P).transpose(0, 3, 2, 1)
    )
    bias2 = np.ascontiguousarray(
        np.broadcast_to(
            np.asarray(bias_param, dtype=np.float32).reshape(1, OUT_F), (P, OUT_F)
        )
    )
    scale2 = np.ascontiguousarray(
        np.asarray(weight_scale, dtype=np.float32).reshape(1, 1)
    )
    in_maps = []
    for c in range(N_CORES):
        in_maps.append(
            {
                "x": np.ascontiguousarray(X[c * TOK_C:(c + 1) * TOK_C]),
                "wt": wp,
                "bias": bias2,
                "scale": scale2,
            }
        )
    return in_maps


def assemble_output(results):
    out = np.concatenate([results[c]["out"] for c in range(N_CORES)], axis=0)
    return np.ascontiguousarray(out.reshape(B, S, OUT_F).astype(np.float32))


def kernel(input, qweight, weight_scale, bias_param):
    from concourse.bass_utils import run_bass_kernel_spmd

    in_maps = prep_inputs(input, qweight, weight_scale, bias_param)
    nc = build_nc()
    res = run_bass_kernel_spmd(nc, in_maps, core_ids=list(range(N_CORES)))
    return assemble_output(res.results)


# revision 9
# speedup vs baseline: 1.0150x; 1.0150x over previous
"""Int8-dequant linear layer (out = input @ (qweight*scale).T + bias) on 8 trn2 cores.

Sharding: token-parallel. input [8,512,4096] flattens to 4096 tokens; each
core computes 512 tokens against the full weight matrix. qweight is repacked
host-side to fp16 (exact: values are integers in [-127,127]) in a
[of_chunk, partition, k_tile, n] layout so each weight DMA is a single
contiguous 4MB read. The per-tensor scale and the bias are applied on-device
in fp32 (ScalarE mul + VectorE add) after fp32 PSUM accumulation, so the only
precision loss vs the fp32 reference is the fp16 rounding of the activations
(~1e-4 relative).
"""

import numpy as np

B, S, IN_F, OUT_F = 8, 512, 4096, 4096
N_CORES = 8
TOK = B * S                # 4096 tokens total
TOK_C = TOK // N_CORES     # 512 tokens per core
P = 128                    # partitions
KT = IN_F // P             # 32 k-tiles
NT = 512                   # out-feature chunk (one fp32 PSUM bank)
OF_CHUNKS = OUT_F // NT    # 8
TT = TOK_C // P            # 4 token tiles per core


def _make_tile_context_cls():
    import bass_rust
    import concourse.mybir as mybir
    from concourse.tile import TileContext, ScopedClock

    class _TC(TileContext):
        # The walrus build in this image rejects more than one semaphore wait
        # per instruction. Split extra waits onto nofuse NOPs committed just
        # before the instruction on the same engine (identical queue
        # semantics: the sequencer blocks on the NOP's wait first).
        def _commit_instruction(self, inst, lazy_reg_writes: bool = True):
            si = getattr(inst, "sync_info", None)
            if (
                si is not None
                and len(si.on_wait) > 1
                and inst.engine != mybir.EngineType.Unassigned
            ):
                waits = list(si.on_wait)
                for i, w in enumerate(waits[:-1]):
                    nop = mybir.InstNoOp(
                        name=f"{inst.name}-ws{i}",
                        sync_info=mybir.SyncInfo(on_wait=[w], on_update=[]),
                        bass_nofuse=True,
                        engine=inst.engine,
                    )
                    self._add_instruction(nop)
                inst.sync_info = mybir.SyncInfo(
                    on_wait=[waits[-1]], on_update=list(si.on_update)
                )
            return super()._commit_instruction(inst, lazy_reg_writes)

        # Same walrus limitation: it can't encode syncs on the exit Drain, so
        # land the end-of-kernel clock waits on single-wait NOPs and use the
        # sequencer-level (EVSEM-only) barrier instead of the drain butterfly.
        def _drain_and_barrier(self, tick_clock, wait_clock):
            nc = self.nc
            carrier = nc.sync.nop(nofuse=True)
            wait_clock.add_sem_waits(
                carrier.ins, ScopedClock({None: tick_clock.global_clock})
            )
            waits = list(carrier.ins.sync_info.on_wait)
            if len(waits) > 1:
                carrier.ins.sync_info = bass_rust.SyncInfo(
                    on_wait=[waits[0]], on_update=[]
                )
                for w in waits[1:]:
                    extra = nc.sync.nop(nofuse=True)
                    extra.ins.sync_info = bass_rust.SyncInfo(
                        on_wait=[w], on_update=[]
                    )
            nc.sync.drain()
            nc.all_engine_barrier(sem_only=True)
            assert self.sems is not None
            popped = nc._tile_sem_poison_stack.pop()
            assert popped is self._sem_poison
            nc.clear_and_free_semaphores(list(self.sems.allocated().values()))
            nc.all_engine_barrier(sem_only=True)

    return _TC


def build_nc():
    """Build the per-core Bass program (SPMD: same program, different x shard)."""
    import concourse.bass as bass
    import concourse.mybir as mybir
    from concourse.masks import make_identity

    f16 = mybir.dt.float16
    f32 = mybir.dt.float32

    nc = bass.Bass("TRN2", target_bir_lowering=False, debug=False)
    x = nc.dram_tensor("x", [TOK_C, IN_F], f32, kind="ExternalInput").ap()
    # weights ship as int8 (exact) and are cast to fp16 inline by the SWDGE
    # DMA — halves weight HBM traffic vs fp16-in-DRAM (measured ~19us/pass).
    wt = nc.dram_tensor(
        "wt", [OF_CHUNKS, P, KT, NT], mybir.dt.int8, kind="ExternalInput"
    ).ap()
    # bias comes pre-broadcast to 128 partitions from the host: a plain
    # contiguous 2MB DMA is ~50us faster than a [1,N]->[128,N] broadcast
    # DMA (which re-reads the same 16KB region 128 times).
    bias = nc.dram_tensor("bias", [P, OUT_F], f32, kind="ExternalInput").ap()
    scale = nc.dram_tensor("scale", [1, 1], f32, kind="ExternalInput").ap()
    out = nc.dram_tensor("out", [TOK_C, OUT_F], f32, kind="ExternalOutput").ap()

    TC = _make_tile_context_cls()
    with TC(nc) as tc:
        with (
            tc.tile_pool(name="persist", bufs=1) as persist,
            tc.tile_pool(name="xstage", bufs=2) as xstage,
            tc.tile_pool(name="wpool", bufs=3) as wpool,
            tc.tile_pool(name="opool", bufs=6) as opool,
            tc.tile_pool(name="pt", bufs=4, space="PSUM") as pt_pool,
            tc.tile_pool(name="pacc", bufs=4, space="PSUM") as pacc_pool,
        ):
            identity = persist.tile([P, P], f16)
            make_identity(nc, identity)
            scale_sb = persist.tile([P, 1], f32)
            nc.sync.dma_start(out=scale_sb, in_=scale.to_broadcast((P, 1)))
            bias_sb = persist.tile([P, OUT_F], f32)
            nc.sync.dma_start(out=bias_sb, in_=bias)

            # xt_all[p, j, t] = x[t, j*128+p]  (fp16 transposed activations)
            xt_all = persist.tile([P, KT, TOK_C], f16)

            # Phase 1: load x (cast fp32->fp16 in DMA), transpose on PE.
            for t in range(TT):
                xs = xstage.tile([P, IN_F], f16)
                if t == 0:
                    # split the first load so transposes (and hence the first
                    # matmuls) start before the whole 2MB cast-DMA lands
                    for q in range(4):
                        nc.gpsimd.dma_start(
                            out=xs[:, q * (IN_F // 4):(q + 1) * (IN_F // 4)],
                            in_=x[t * P:(t + 1) * P,
                                  q * (IN_F // 4):(q + 1) * (IN_F // 4)],
                        )
                else:
                    nc.gpsimd.dma_start(out=xs, in_=x[t * P:(t + 1) * P, :])
                for j in range(KT):
                    ps = pt_pool.tile([P, P], f16)
                    nc.tensor.transpose(ps, xs[:, j * P:(j + 1) * P], identity)
                    nc.vector.tensor_copy(xt_all[:, j, t * P:(t + 1) * P], ps)

            # Phase 2: stream weight chunks, accumulate, epilogue, store.
            for of in range(OF_CHUNKS):
                wc = wpool.tile([P, KT, NT], f16)
                if of == 0:
                    # same split for the first weight chunk: matmul j can start
                    # once its k-block is resident
                    for q in range(4):
                        nc.gpsimd.dma_start(
                            out=wc[:, q * (KT // 4):(q + 1) * (KT // 4), :],
                            in_=wt[of, :, q * (KT // 4):(q + 1) * (KT // 4), :],
                        )
                else:
                    nc.gpsimd.dma_start(out=wc, in_=wt[of])  # int8 -> fp16 cast
                for t in range(TT):
                    acc = pacc_pool.tile([P, NT], f32)
                    for j in range(KT):
                        nc.tensor.matmul(
                            acc,
                            lhsT=xt_all[:, j, t * P:(t + 1) * P],
                            rhs=wc[:, j, :],
                            start=(j == 0),
                            stop=(j == KT - 1),
                        )
                    osb = opool.tile([P, NT], f32)
                    nc.scalar.mul(osb, acc, scale_sb[:, :])
                    nc.vector.tensor_add(osb, osb, bias_sb[:, of * NT:(of + 1) * NT])
                    nc.sync.dma_start(
                        out=out[t * P:(t + 1) * P, of * NT:(of + 1) * NT], in_=osb
                    )
    return nc


def prep_inputs(input, qweight, weight_scale, bias_param):
    """Host-side shard/repack. Returns per-core in_maps."""
    X = np.ascontiguousarray(np.asarray(input, dtype=np.float32).reshape(TOK, IN_F))
    # int8 container for the int8-valued weights; the device DMA casts to fp16
    # (exact for integers in [-127,127]).
    q8 = np.asarray(qweight).astype(np.int8)
    # w_packed[of, p, j, n] = qweight[of*NT + n, j*P + p]
    wp = np.ascontiguousarray(
        q8.reshape(OF_CHUNKS, NT, KT, P).transpose(0, 3, 2, 1)
    )
    bias2 = np.ascontiguousarray(
        np.broadcast_to(
            np.asarray(bias_param, dtype=np.float32).reshape(1, OUT_F), (P, OUT_F)
        )
    )
    scale2 = np.ascontiguousarray(
        np.asarray(weight_scale, dtype=np.float32).reshape(1, 1)
    )
    in_maps = []
    for c in range(N_CORES):
        in_maps.append(
            {
                "x": np.ascontiguousarray(X[c * TOK_C:(c + 1) * TOK_C]),
                "wt": wp,
                "bias": bias2,
                "scale": scale2,
            }
        )
    return in_maps


def assemble_output(results):
    out = np.concatenate([results[c]["out"] for c in range(N_CORES)], axis=0)
    return np.ascontiguousarray(out.reshape(B, S, OUT_F).astype(np.float32))


def kernel(input, qweight, weight_scale, bias_param):
    from concourse.bass_utils import run_bass_kernel_spmd

    in_maps = prep_inputs(input, qweight, weight_scale, bias_param)
    nc = build_nc()
    res = run_bass_kernel_spmd(nc, in_maps, core_ids=list(range(N_CORES)))
    return assemble_output(res.results)
